# revision 6
# baseline (speedup 1.0000x reference)
"""Multi-head attention (B=4, T=2048, D=1024, H=16) on 8 TRN2 NeuronCores.

Sharding: core c -> (batch b = c//2, head-group g = c%2 of 8 heads).
Each core computes the qkv projection for its batch restricted to its 8
heads, full attention for those heads, and a partial output projection
(ctx_local @ Wout[rows of its heads]).  Host sums the two partials per batch.

Per-core kernel: a single flat software pipeline over 256 attention chunks
(4 head-pairs x 4 query-quarters x 16 k-chunks).  Per chunk: an S pair (two
concurrent 64-row-tile matmuls, one per head), exp on ACT, and the AV pair
deferred 3 chunks.  All projection work (qk for later pairs, v, output) is
broken into ~215ns steps and scheduled into specific chunks so the PE stream
stays ahead of ACT, which is the binding engine (~1us per chunk).  x stays
resident in SBUF; DMAs are issued in first-use order so attention starts
~11us in instead of ~43us.
"""

import numpy as np
import ml_dtypes
from contextlib import ExitStack

import concourse.bass as bass
import concourse.bacc as bacc
import concourse.tile as tile
from concourse import mybir
from concourse.bass_utils import run_bass_kernel_spmd

FP32 = mybir.dt.float32
BF16 = mybir.dt.bfloat16
EXP = mybir.ActivationFunctionType.Exp

D = 1024
T = 2048
HPC = 8          # heads per core
FC = 8           # feature chunks of 128 (projection contraction)
KC = 16          # k chunks of 128 per quarter
NG = 256         # total chunks: 4 pairs x 4 quarters x 16
AVD = 3          # AV defer (chunks)


def _norm(nc, rpool, ctx_sb, ctxp, hh, hc, qsl):
    """ctx_sb[hb:hb+64, hc, qsl] = ctxp[0:64] / ctxp[64] (sumexp row)."""
    hb = (hh % 2) * 64
    rtmp = rpool.tile([1, 512], FP32, tag="rtmp")
    nc.vector.tensor_copy(out=rtmp[:], in_=ctxp[64:65, :])
    rt = rpool.tile([1, 512], FP32, tag="rt")
    nc.vector.reciprocal_approx_fast(out=rt[:], in_=rtmp[:])
    rb = rpool.tile([64, 512], FP32, tag="rb")
    nc.gpsimd.partition_broadcast(rb[:], rt[0:1, :], channels=64)
    nc.vector.tensor_mul(ctx_sb[hb:hb + 64, hc, qsl], ctxp[0:64, :], rb[:])


def _body(ctx, nc, tc, xt_d, wq_d, wk_d, wv_d, wo_d, out_d):
    xt_r = xt_d.rearrange("(f p) t -> p f t", p=128)
    persist = ctx.enter_context(tc.tile_pool(name="persist", bufs=1))
    xbig = persist.tile([128, FC, T], BF16, tag="x")
    qT = persist.tile([128, 4, T], BF16, tag="qT")
    kT = persist.tile([128, 4, T], BF16, tag="kT")
    v_sb = persist.tile([128, KC, HPC, 65], BF16, tag="v")
    ctx_sb = persist.tile([128, 4, T], BF16, tag="ctx")
    wq_sb = persist.tile([128, FC, 512], BF16, tag="wq")
    wk_sb = persist.tile([128, FC, 512], BF16, tag="wk")
    wv_sb = persist.tile([128, FC, 512], BF16, tag="wv")
    wo_sb = persist.tile([128, 4, D], BF16, tag="wo")
    warm = persist.tile([1, 4], FP32, tag="warm")

    # Preload the ACT exp table-set during the initial DMA wait.
    nc.vector.memset(warm[:], 0.0)
    nc.scalar.activation(out=warm[:], in_=warm[:], func=EXP)

    nc.vector.memset(v_sb[:, :, :, 64:65], 1.0)

    # DMA in first-use order: wq+x0 (span-0 q proj), wk (span-0 k proj),
    # wv lo-half (v heads 0-3 JIT in the first quarter), x spans 1-3
    # (k-span proj at chunks 0-11), wv hi-half, wout (used from g=208).
    nc.sync.dma_start(out=wq_sb[:], in_=wq_d.rearrange("(f p) c -> p f c", p=128))
    nc.sync.dma_start(out=xbig[:, :, 0:512], in_=xt_r[:, :, 0:512])
    nc.sync.dma_start(out=wk_sb[:], in_=wk_d.rearrange("(f p) c -> p f c", p=128))
    wv_r = wv_d.rearrange("(f p) c -> p f c", p=128)
    nc.sync.dma_start(out=wv_sb[:, :, 0:256], in_=wv_r[:, :, 0:256])
    nc.sync.dma_start(out=xbig[:, :, 512:1024], in_=xt_r[:, :, 512:1024])
    nc.sync.dma_start(out=xbig[:, :, 1024:1536], in_=xt_r[:, :, 1024:1536])
    nc.sync.dma_start(out=wv_sb[:, :, 256:512], in_=wv_r[:, :, 256:512])
    nc.sync.dma_start(out=xbig[:, :, 1536:2048], in_=xt_r[:, :, 1536:2048])
    nc.sync.dma_start(out=wo_sb[:], in_=wo_d.rearrange("(c p) d -> p c d", p=128))

    spsum = ctx.enter_context(tc.tile_pool(name="spsum", bufs=2, space="PSUM"))
    cpsum = ctx.enter_context(tc.tile_pool(name="cpsum", bufs=2, space="PSUM"))
    # 8 PSUM banks total: spsum 4, ctx ring 2, one shared qk/op accumulator
    # bank (lifetimes are disjoint / cleanly sequential), one v bank.
    ps_qk = ctx.enter_context(tc.tile_pool(name="ps_qk", bufs=1, space="PSUM"))
    ps_v = ctx.enter_context(tc.tile_pool(name="ps_v", bufs=1, space="PSUM"))
    ppool = ctx.enter_context(tc.tile_pool(name="P2", bufs=8))
    rpool = ctx.enter_context(tc.tile_pool(name="rpool", bufs=2))
    osb = ctx.enter_context(tc.tile_pool(name="osb", bufs=2))

    # ---- projection step machinery ----

    def qk_burst(pair, ts, qk):
        """Immediate 8-matmul projection of one (pair, span, q-or-k)."""
        tsl = slice(ts * 512, (ts + 1) * 512)
        w_sb, dst = ((wq_sb, qT), (wk_sb, kT))[qk]
        p = ps_qk.tile([128, 512], FP32, tag="qkp")
        for fc in range(FC):
            nc.tensor.matmul(
                p[:],
                lhsT=w_sb[:, fc, pair * 128:(pair + 1) * 128],
                rhs=xbig[:, fc, tsl],
                start=(fc == 0), stop=(fc == FC - 1))
        nc.vector.tensor_copy(out=dst[:, pair, tsl], in_=p[:])

    def make_qk_stepper(pair, units):
        """One-matmul steps; ``units`` is a list of (ts, qk) in need-order."""
        st = {"p": None}

        def step(s):
            unit, fc = divmod(s, FC)
            ts, qk = units[unit]
            tsl = slice(ts * 512, (ts + 1) * 512)
            w_sb, dst = ((wq_sb, qT), (wk_sb, kT))[qk]
            if fc == 0:
                st["p"] = ps_qk.tile([128, 512], FP32, tag="qkp", name="qkp")
            nc.tensor.matmul(
                st["p"][:],
                lhsT=w_sb[:, fc, pair * 128:(pair + 1) * 128],
                rhs=xbig[:, fc, tsl],
                start=(fc == 0), stop=(fc == FC - 1))
            if fc == FC - 1:
                nc.vector.tensor_copy(out=dst[:, pair, tsl], in_=st["p"][:])
        return step

    def v_unit(kc, lo):
        """v projection for k-chunk kc, heads 0-3 (lo) or 4-7 (hi)."""
        csl = slice(0, 256) if lo else slice(256, 512)
        psv = ps_v.tile([128, 256], FP32, tag="vps")
        xc = slice((kc // 4) * 512 + (kc % 4) * 128,
                   (kc // 4) * 512 + (kc % 4) * 128 + 128)
        for fc in range(FC):
            nc.tensor.matmul(
                psv[:],
                lhsT=xbig[:, fc, xc],
                rhs=wv_sb[:, fc, csl],
                start=(fc == 0), stop=(fc == FC - 1))
        h0 = 0 if lo else 4
        nc.vector.tensor_copy(
            out=v_sb[:, kc, h0:h0 + 4, 0:64],
            in_=psv[:].rearrange("p (h d) -> p h d", h=4))

    def make_op_steps(qq_prev):
        """16 steps emitting the output projection of qq_prev's tokens
        (4 token chunks x 2 column halves x accumulate 4 cc)."""
        st = {"po": None, "ot": None}

        def step(s):
            unit, half = divmod(s, 2)
            tcg = qq_prev * 4 + unit // 2
            j2 = unit % 2
            if half == 0:
                if j2 == 0:
                    st["ot"] = osb.tile([128, D], FP32, tag="ot", name="ot")
                st["po"] = ps_qk.tile([128, 512], FP32, tag="qkp", name="po")
                ccs = (0, 1)
            else:
                ccs = (2, 3)
            for cc in ccs:
                nc.tensor.matmul(
                    st["po"][:],
                    lhsT=ctx_sb[:, cc, tcg * 128:(tcg + 1) * 128],
                    rhs=wo_sb[:, cc, j2 * 512:(j2 + 1) * 512],
                    start=(cc == 0), stop=(cc == 3))
            if half == 1:
                nc.vector.tensor_copy(
                    out=st["ot"][:, j2 * 512:(j2 + 1) * 512], in_=st["po"][:])
                if j2 == 1:
                    nc.sync.dma_start(
                        out=out_d[tcg * 128:(tcg + 1) * 128, :],
                        in_=st["ot"][:])
        return step

    # ---- schedule: extra PE work per global chunk g ----

    sched = {g: [] for g in range(NG)}

    def put(g, fn, *args):
        sched[g].append((fn, args))

    # pair 0 remaining spans: k1,k2,k3 (needed by g=4,8,12), q1,q2,q3
    # (needed by g=16,32,48) -- 48 steps, 2 per chunk over g 0-23.
    p0_step = make_qk_stepper(
        0, [(1, 1), (2, 1), (3, 1), (1, 0), (2, 0), (3, 0)])
    for s in range(48):
        put(s // 2, p0_step, s)
    # pairs 1-3: k0..k3 then q0..q3, 64 steps, 2 per chunk over 32 chunks
    # ending 8 chunks before the pair's first attention chunk.
    for pair in range(1, 4):
        stp = make_qk_stepper(
            pair, [(0, 1), (1, 1), (2, 1), (3, 1),
                   (0, 0), (1, 0), (2, 0), (3, 0)])
        g0 = 64 * pair - 40
        for s in range(64):
            put(g0 + s // 2, stp, s)
    # v heads 0-3: JIT, chunk kc (AV(kc) is emitted at chunk kc+AVD).
    for kc in range(KC):
        put(kc, v_unit, kc, True)
    # v heads 4-7: needed by g=64; spread over g 26..57.
    for kc in range(KC):
        put(26 + 2 * kc, v_unit, kc, False)
    # output projection: quarter Q's 16 steps inside hc3's quarter Q+1,
    # chunks 4..11 (2 per chunk), after quarter Q's norm has landed.
    for q in range(3):
        ops = make_op_steps(q)
        for s in range(16):
            put(208 + 16 * q + 4 + s // 2, ops, s)

    # ---- the flat attention pipeline ----

    qk_burst(0, 0, 0)
    qk_burst(0, 0, 1)

    qinfo = {}

    def emit_av(j):
        info = qinfo[j // 16]
        kc = j % 16
        p2 = info["P2"][kc]
        for i, ctxp in ((0, info["ctxA"]), (1, info["ctxB"])):
            nc.tensor.matmul(
                ctxp[:],
                lhsT=v_sb[:, kc, 2 * info["hc"] + i, :],
                rhs=p2[:, i, :],
                start=(kc == 0), stop=(kc == KC - 1))

    def emit_norm(q):
        info = qinfo[q]
        qsl = slice(info["qq"] * 512, (info["qq"] + 1) * 512)
        _norm(nc, rpool, ctx_sb, info["ctxA"], 2 * info["hc"], info["hc"], qsl)
        _norm(nc, rpool, ctx_sb, info["ctxB"], 2 * info["hc"] + 1,
              info["hc"], qsl)
        del qinfo[q]

    for g in range(NG):
        hc, qq, kc = g // 64, (g // 16) % 4, g % 16
        if kc == 0:
            qinfo[g // 16] = {
                "hc": hc, "qq": qq, "P2": [],
                "ctxA": cpsum.tile([65, 512], FP32, tag="ctx", name="ctxA"),
                "ctxB": cpsum.tile([65, 512], FP32, tag="ctx", name="ctxB"),
            }
        qsl = slice(qq * 512, (qq + 1) * 512)
        sps = spsum.tile([128, 2, 512], FP32, tag="S")
        for i in range(2):          # head A on rows 0-63, head B on 64-127
            b0 = i * 64
            nc.tensor.matmul(
                sps[:, i, :],
                lhsT=kT[b0:b0 + 64, hc, kc * 128:(kc + 1) * 128],
                rhs=qT[b0:b0 + 64, hc, qsl],
                start=True, stop=True)
        p2 = ppool.tile([128, 2, 512], BF16, tag="P2")
        nc.scalar.activation(out=p2[:], in_=sps[:], func=EXP, scale=0.125)
        qinfo[g // 16]["P2"].append(p2)
        for fn, args in sched[g]:
            fn(*args)
        if g >= AVD:
            emit_av(g - AVD)
            if (g - AVD) % 16 == 15:
                emit_norm((g - AVD) // 16)
    for j in range(NG - AVD, NG):
        emit_av(j)
    emit_norm(15)
    # tail: output projection for the last quarter
    ops = make_op_steps(3)
    for s in range(16):
        ops(s)


def build():
    nc = bacc.Bacc("TRN2", target_bir_lowering=False, debug=False, num_devices=8)
    xt_d = nc.dram_tensor("xt", [D, T], BF16, kind="ExternalInput").ap()
    wq_d = nc.dram_tensor("wq", [D, 512], BF16, kind="ExternalInput").ap()
    wk_d = nc.dram_tensor("wk", [D, 512], BF16, kind="ExternalInput").ap()
    wv_d = nc.dram_tensor("wv", [D, 512], BF16, kind="ExternalInput").ap()
    wo_d = nc.dram_tensor("wout", [512, D], BF16, kind="ExternalInput").ap()
    out_d = nc.dram_tensor("out", [T, D], FP32, kind="ExternalOutput").ap()
    with tile.TileContext(nc) as tc:
        with ExitStack() as ctx:
            _body(ctx, nc, tc, xt_d, wq_d, wk_d, wv_d, wo_d, out_d)
    nc.compile()
    return nc


_nc = None


def _get_nc():
    global _nc
    if _nc is None:
        _nc = build()
    return _nc


def make_in_maps(x, Wqkv, Wout):
    bf = ml_dtypes.bfloat16
    in_maps = []
    for c in range(8):
        b, g = divmod(c, 2)
        cs = slice(g * 512, (g + 1) * 512)
        in_maps.append({
            "xt": np.ascontiguousarray(x[b].T).astype(bf),
            "wq": np.ascontiguousarray(Wqkv[:, 0 * D:1 * D][:, cs]).astype(bf),
            "wk": np.ascontiguousarray(Wqkv[:, 1 * D:2 * D][:, cs]).astype(bf),
            "wv": np.ascontiguousarray(Wqkv[:, 2 * D:3 * D][:, cs]).astype(bf),
            "wout": np.ascontiguousarray(Wout[cs, :]).astype(bf),
        })
    return in_maps


def kernel(x, Wqkv, Wout, _trace=False):
    nc = _get_nc()
    x = np.asarray(x, dtype=np.float32)
    Wqkv = np.asarray(Wqkv, dtype=np.float32)
    Wout = np.asarray(Wout, dtype=np.float32)
    in_maps = make_in_maps(x, Wqkv, Wout)
    kwargs = {}
    if _trace:
        kwargs["trace"] = True
    res = run_bass_kernel_spmd(nc, in_maps, core_ids=list(range(8)), **kwargs)
    outs = [res.results[c]["out"] for c in range(8)]
    out = np.stack([outs[2 * b] + outs[2 * b + 1] for b in range(4)])
    if _trace:
        kernel.last_result = res
    return out


# revision 8
# speedup vs baseline: 1.1945x; 1.1945x over previous
"""Multi-head attention (B=4, T=2048, D=1024, H=16) on 8 TRN2 NeuronCores.

Sharding: core c -> (batch b = c//2, head-group g = c%2 of 8 heads).
Each core computes the qkv projection for its batch restricted to its 8
heads, full attention for those heads, and a partial output projection
(ctx_local @ Wout[rows of its heads]).  Host sums the two partials per batch.

Per-core kernel: a single flat software pipeline over 256 attention chunks
(4 head-pairs x 4 query-quarters x 16 k-chunks).  Per chunk: an S pair (two
concurrent 64-row-tile matmuls, one per head), exp on ACT, and the AV pair
deferred 3 chunks.  All projection work (qk for later pairs, v, output) is
broken into ~215ns steps and scheduled into specific chunks so the PE stream
stays ahead of ACT, which is the binding engine (~1us per chunk).  x stays
resident in SBUF; DMAs are issued in first-use order so attention starts
~11us in instead of ~43us.
"""

import numpy as np
import ml_dtypes
from contextlib import ExitStack

import concourse.bass as bass
import concourse.bacc as bacc
import concourse.tile as tile
from concourse import mybir
from concourse.bass_utils import run_bass_kernel_spmd

FP32 = mybir.dt.float32
BF16 = mybir.dt.bfloat16
EXP = mybir.ActivationFunctionType.Exp

D = 1024
T = 2048
HPC = 8          # heads per core
FC = 8           # feature chunks of 128 (projection contraction)
KC = 16          # k chunks of 128 per quarter
NG = 256         # total chunks: 4 pairs x 4 quarters x 16
AVD = 3          # AV defer (chunks)


def _norm(nc, rpool, ctx_sb, ctxp, hh, hc, qsl):
    """ctx_sb[hb:hb+64, hc, qsl] = ctxp[0:64] / ctxp[64] (sumexp row)."""
    hb = (hh % 2) * 64
    rtmp = rpool.tile([1, 512], FP32, tag="rtmp")
    nc.vector.tensor_copy(out=rtmp[:], in_=ctxp[64:65, :])
    rt = rpool.tile([1, 512], FP32, tag="rt")
    nc.vector.reciprocal_approx_fast(out=rt[:], in_=rtmp[:])
    rb = rpool.tile([64, 512], FP32, tag="rb")
    nc.gpsimd.partition_broadcast(rb[:], rt[0:1, :], channels=64)
    nc.vector.tensor_mul(ctx_sb[hb:hb + 64, hc, qsl], ctxp[0:64, :], rb[:])


def _body(ctx, nc, tc, xt_d, wq_d, wk_d, wv_d, wo_d, out_d):
    xt_r = xt_d.rearrange("(f p) t -> p f t", p=128)
    persist = ctx.enter_context(tc.tile_pool(name="persist", bufs=1))
    xbig = persist.tile([128, FC, T], BF16, tag="x")
    qT = persist.tile([128, 4, T], BF16, tag="qT")
    kT = persist.tile([128, 4, T], BF16, tag="kT")
    v_sb = persist.tile([128, KC, HPC, 65], BF16, tag="v")
    ctx_sb = persist.tile([128, 4, T], BF16, tag="ctx")
    wq_sb = persist.tile([128, FC, 512], BF16, tag="wq")
    wk_sb = persist.tile([128, FC, 512], BF16, tag="wk")
    wv_sb = persist.tile([128, FC, 512], BF16, tag="wv")
    wo_sb = persist.tile([128, 4, D], BF16, tag="wo")
    # circular exp-output buffer, indexed by global chunk mod NP2; reuse
    # distance (20 chunks) far exceeds the AV defer + quarter-trailing window.
    P2big = persist.tile([128, 20, 2, 512], BF16, tag="P2big")
    warm = persist.tile([1, 4], FP32, tag="warm")

    # Preload the ACT exp table-set during the initial DMA wait.
    nc.vector.memset(warm[:], 0.0)
    nc.scalar.activation(out=warm[:], in_=warm[:], func=EXP)

    nc.vector.memset(v_sb[:, :, :, 64:65], 1.0)

    # DMA in first-use order: wq+x0 (span-0 q proj), wk (span-0 k proj),
    # wv lo-half (v heads 0-3 JIT in the first quarter), x spans 1-3
    # (k-span proj at chunks 0-11), wv hi-half, wout (used from g=208).
    nc.sync.dma_start(out=wq_sb[:], in_=wq_d.rearrange("(f p) c -> p f c", p=128))
    nc.sync.dma_start(out=xbig[:, :, 0:512], in_=xt_r[:, :, 0:512])
    nc.sync.dma_start(out=wk_sb[:], in_=wk_d.rearrange("(f p) c -> p f c", p=128))
    wv_r = wv_d.rearrange("(f p) c -> p f c", p=128)
    nc.sync.dma_start(out=wv_sb[:, :, 0:256], in_=wv_r[:, :, 0:256])
    nc.sync.dma_start(out=xbig[:, :, 512:1024], in_=xt_r[:, :, 512:1024])
    nc.sync.dma_start(out=xbig[:, :, 1024:1536], in_=xt_r[:, :, 1024:1536])
    nc.sync.dma_start(out=wv_sb[:, :, 256:512], in_=wv_r[:, :, 256:512])
    nc.sync.dma_start(out=xbig[:, :, 1536:2048], in_=xt_r[:, :, 1536:2048])
    nc.sync.dma_start(out=wo_sb[:], in_=wo_d.rearrange("(c p) d -> p c d", p=128))

    spsum = ctx.enter_context(tc.tile_pool(name="spsum", bufs=2, space="PSUM"))
    cpsum = ctx.enter_context(tc.tile_pool(name="cpsum", bufs=2, space="PSUM"))
    # 8 PSUM banks total: spsum 4, ctx ring 2, one shared qk/op accumulator
    # bank (lifetimes are disjoint / cleanly sequential), one v bank.
    ps_qk = ctx.enter_context(tc.tile_pool(name="ps_qk", bufs=1, space="PSUM"))
    ps_v = ctx.enter_context(tc.tile_pool(name="ps_v", bufs=1, space="PSUM"))
    rpool = ctx.enter_context(tc.tile_pool(name="rpool", bufs=2))
    osb = ctx.enter_context(tc.tile_pool(name="osb", bufs=2))

    # ---- projection step machinery ----

    def qk_burst(pair, ts, qk):
        """Immediate 8-matmul projection of one (pair, span, q-or-k)."""
        tsl = slice(ts * 512, (ts + 1) * 512)
        w_sb, dst = ((wq_sb, qT), (wk_sb, kT))[qk]
        p = ps_qk.tile([128, 512], FP32, tag="qkp")
        for fc in range(FC):
            nc.tensor.matmul(
                p[:],
                lhsT=w_sb[:, fc, pair * 128:(pair + 1) * 128],
                rhs=xbig[:, fc, tsl],
                start=(fc == 0), stop=(fc == FC - 1))
        nc.vector.tensor_copy(out=dst[:, pair, tsl], in_=p[:])

    def make_qk_stepper(pair, units):
        """One-matmul steps; ``units`` is a list of (ts, qk) in need-order."""
        st = {"p": None}

        def step(s):
            unit, fc = divmod(s, FC)
            ts, qk = units[unit]
            tsl = slice(ts * 512, (ts + 1) * 512)
            w_sb, dst = ((wq_sb, qT), (wk_sb, kT))[qk]
            if fc == 0:
                st["p"] = ps_qk.tile([128, 512], FP32, tag="qkp", name="qkp")
            nc.tensor.matmul(
                st["p"][:],
                lhsT=w_sb[:, fc, pair * 128:(pair + 1) * 128],
                rhs=xbig[:, fc, tsl],
                start=(fc == 0), stop=(fc == FC - 1))
            if fc == FC - 1:
                nc.vector.tensor_copy(out=dst[:, pair, tsl], in_=st["p"][:])
        return step

    def v_unit(kc, lo):
        """v projection for k-chunk kc, heads 0-3 (lo) or 4-7 (hi)."""
        csl = slice(0, 256) if lo else slice(256, 512)
        psv = ps_v.tile([128, 256], FP32, tag="vps")
        xc = slice((kc // 4) * 512 + (kc % 4) * 128,
                   (kc // 4) * 512 + (kc % 4) * 128 + 128)
        for fc in range(FC):
            nc.tensor.matmul(
                psv[:],
                lhsT=xbig[:, fc, xc],
                rhs=wv_sb[:, fc, csl],
                start=(fc == 0), stop=(fc == FC - 1))
        h0 = 0 if lo else 4
        nc.vector.tensor_copy(
            out=v_sb[:, kc, h0:h0 + 4, 0:64],
            in_=psv[:].rearrange("p (h d) -> p h d", h=4))

    def make_op_steps(qq_prev):
        """16 steps emitting the output projection of qq_prev's tokens
        (4 token chunks x 2 column halves x accumulate 4 cc)."""
        st = {"po": None, "ot": None}

        def step(s):
            unit, half = divmod(s, 2)
            tcg = qq_prev * 4 + unit // 2
            j2 = unit % 2
            if half == 0:
                if j2 == 0:
                    st["ot"] = osb.tile([128, D], FP32, tag="ot", name="ot")
                st["po"] = ps_qk.tile([128, 512], FP32, tag="qkp", name="po")
                ccs = (0, 1)
            else:
                ccs = (2, 3)
            for cc in ccs:
                nc.tensor.matmul(
                    st["po"][:],
                    lhsT=ctx_sb[:, cc, tcg * 128:(tcg + 1) * 128],
                    rhs=wo_sb[:, cc, j2 * 512:(j2 + 1) * 512],
                    start=(cc == 0), stop=(cc == 3))
            if half == 1:
                nc.vector.tensor_copy(
                    out=st["ot"][:, j2 * 512:(j2 + 1) * 512], in_=st["po"][:])
                if j2 == 1:
                    nc.sync.dma_start(
                        out=out_d[tcg * 128:(tcg + 1) * 128, :],
                        in_=st["ot"][:])
        return step

    # ---- schedule: extra PE work per global chunk g ----

    sched = {g: [] for g in range(NG)}

    def put(g, fn, *args):
        sched[g].append((fn, args))

    # pair 0 remaining spans: k1,k2,k3 (needed by g=4,8,12), q1,q2,q3
    # (needed by g=16,32,48) -- 48 steps, 2 per chunk over g 0-23.
    p0_step = make_qk_stepper(
        0, [(1, 1), (2, 1), (3, 1), (1, 0), (2, 0), (3, 0)])
    for s in range(48):
        put(s // 2, p0_step, s)
    # pairs 1-3: k0..k3 then q0..q3, 64 steps, 2 per chunk over 32 chunks
    # ending 8 chunks before the pair's first attention chunk.
    for pair in range(1, 4):
        stp = make_qk_stepper(
            pair, [(0, 1), (1, 1), (2, 1), (3, 1),
                   (0, 0), (1, 0), (2, 0), (3, 0)])
        g0 = 64 * pair - 40
        for s in range(64):
            put(g0 + s // 2, stp, s)
    # v heads 0-3: JIT, chunk kc (AV(kc) is emitted at chunk kc+AVD).
    for kc in range(KC):
        put(kc, v_unit, kc, True)
    # v heads 4-7: needed by g=64; spread over g 26..57.
    for kc in range(KC):
        put(26 + 2 * kc, v_unit, kc, False)
    # output projection: quarter Q's 16 steps inside hc3's quarter Q+1,
    # chunks 4..11 (2 per chunk), after quarter Q's norm has landed.
    for q in range(3):
        ops = make_op_steps(q)
        for s in range(16):
            put(208 + 16 * q + 4 + s // 2, ops, s)

    # ---- the flat attention pipeline ----

    qk_burst(0, 0, 0)
    qk_burst(0, 0, 1)

    qinfo = {}

    def emit_av(j):
        info = qinfo[j // 16]
        kc = j % 16
        for i, ctxp in ((0, info["ctxA"]), (1, info["ctxB"])):
            nc.tensor.matmul(
                ctxp[:],
                lhsT=v_sb[:, kc, 2 * info["hc"] + i, :],
                rhs=P2big[:, j % 20, i, :],
                start=(kc == 0), stop=(kc == KC - 1))

    def emit_norm(q):
        info = qinfo[q]
        qsl = slice(info["qq"] * 512, (info["qq"] + 1) * 512)
        _norm(nc, rpool, ctx_sb, info["ctxA"], 2 * info["hc"], info["hc"], qsl)
        _norm(nc, rpool, ctx_sb, info["ctxB"], 2 * info["hc"] + 1,
              info["hc"], qsl)
        del qinfo[q]

    for g in range(NG):
        hc, qq, kc = g // 64, (g // 16) % 4, g % 16
        if kc == 0:
            qinfo[g // 16] = {
                "hc": hc, "qq": qq,
                "ctxA": cpsum.tile([65, 512], FP32, tag="ctx", name="ctxA"),
                "ctxB": cpsum.tile([65, 512], FP32, tag="ctx", name="ctxB"),
            }
        qsl = slice(qq * 512, (qq + 1) * 512)
        sps = spsum.tile([128, 2, 512], FP32, tag="S")
        for i in range(2):          # head A on rows 0-63, head B on 64-127
            b0 = i * 64
            nc.tensor.matmul(
                sps[:, i, :],
                lhsT=kT[b0:b0 + 64, hc, kc * 128:(kc + 1) * 128],
                rhs=qT[b0:b0 + 64, hc, qsl],
                start=True, stop=True)
        nc.scalar.activation(
            out=P2big[:, g % 20, :, :], in_=sps[:], func=EXP, scale=0.125)
        for fn, args in sched[g]:
            fn(*args)
        if g >= AVD:
            emit_av(g - AVD)
            if (g - AVD) % 16 == 15:
                emit_norm((g - AVD) // 16)
    for j in range(NG - AVD, NG):
        emit_av(j)
    emit_norm(15)
    # tail: output projection for the last quarter
    ops = make_op_steps(3)
    for s in range(16):
        ops(s)


def build():
    nc = bacc.Bacc("TRN2", target_bir_lowering=False, debug=False, num_devices=8)
    xt_d = nc.dram_tensor("xt", [D, T], BF16, kind="ExternalInput").ap()
    wq_d = nc.dram_tensor("wq", [D, 512], BF16, kind="ExternalInput").ap()
    wk_d = nc.dram_tensor("wk", [D, 512], BF16, kind="ExternalInput").ap()
    wv_d = nc.dram_tensor("wv", [D, 512], BF16, kind="ExternalInput").ap()
    wo_d = nc.dram_tensor("wout", [512, D], BF16, kind="ExternalInput").ap()
    out_d = nc.dram_tensor("out", [T, D], FP32, kind="ExternalOutput").ap()
    with tile.TileContext(nc) as tc:
        with ExitStack() as ctx:
            _body(ctx, nc, tc, xt_d, wq_d, wk_d, wv_d, wo_d, out_d)
    nc.compile()
    return nc


_nc = None


def _get_nc():
    global _nc
    if _nc is None:
        _nc = build()
    return _nc


def make_in_maps(x, Wqkv, Wout):
    bf = ml_dtypes.bfloat16
    in_maps = []
    for c in range(8):
        b, g = divmod(c, 2)
        cs = slice(g * 512, (g + 1) * 512)
        in_maps.append({
            "xt": np.ascontiguousarray(x[b].T).astype(bf),
            "wq": np.ascontiguousarray(Wqkv[:, 0 * D:1 * D][:, cs]).astype(bf),
            "wk": np.ascontiguousarray(Wqkv[:, 1 * D:2 * D][:, cs]).astype(bf),
            "wv": np.ascontiguousarray(Wqkv[:, 2 * D:3 * D][:, cs]).astype(bf),
            "wout": np.ascontiguousarray(Wout[cs, :]).astype(bf),
        })
    return in_maps


def kernel(x, Wqkv, Wout, _trace=False):
    nc = _get_nc()
    x = np.asarray(x, dtype=np.float32)
    Wqkv = np.asarray(Wqkv, dtype=np.float32)
    Wout = np.asarray(Wout, dtype=np.float32)
    in_maps = make_in_maps(x, Wqkv, Wout)
    kwargs = {}
    if _trace:
        kwargs["trace"] = True
    res = run_bass_kernel_spmd(nc, in_maps, core_ids=list(range(8)), **kwargs)
    outs = [res.results[c]["out"] for c in range(8)]
    out = np.stack([outs[2 * b] + outs[2 * b + 1] for b in range(4)])
    if _trace:
        kernel.last_result = res
    return out


# revision 10
# speedup vs baseline: 1.2131x; 1.0156x over previous
"""Multi-head attention (B=4, T=2048, D=1024, H=16) on 8 TRN2 NeuronCores.

Sharding: core c -> (batch b = c//2, head-group g = c%2 of 8 heads).
Each core computes the qkv projection for its batch restricted to its 8
heads, full attention for those heads, and a partial output projection
(ctx_local @ Wout[rows of its heads]).  Host sums the two partials per batch.

Per-core kernel: a single flat software pipeline over 256 attention chunks
(4 head-pairs x 4 query-quarters x 16 k-chunks).  Per chunk: an S pair (two
concurrent 64-row-tile matmuls, one per head), exp on ACT, and the AV pair
deferred 3 chunks.  All projection work (qk for later pairs, v, output) is
broken into ~215ns steps and scheduled into specific chunks so the PE stream
stays ahead of ACT, which is the binding engine (~1us per chunk).  x stays
resident in SBUF; DMAs are issued in first-use order so attention starts
~11us in instead of ~43us.
"""

import numpy as np
import ml_dtypes
from contextlib import ExitStack

import concourse.bass as bass
import concourse.bacc as bacc
import concourse.tile as tile
from concourse import mybir
from concourse.bass_utils import run_bass_kernel_spmd

FP32 = mybir.dt.float32
BF16 = mybir.dt.bfloat16
EXP = mybir.ActivationFunctionType.Exp

D = 1024
T = 2048
HPC = 8          # heads per core
FC = 8           # feature chunks of 128 (projection contraction)
KC = 16          # k chunks of 128 per quarter
NG = 256         # total chunks: 4 pairs x 4 quarters x 16
AVD = 3          # AV defer (chunks)


def _norm(nc, rpool, ctx_sb, ctxp, hh, hc, qsl):
    """ctx_sb[hb:hb+64, hc, qsl] = ctxp[0:64] / ctxp[64] (sumexp row)."""
    hb = (hh % 2) * 64
    rtmp = rpool.tile([1, 512], FP32, tag="rtmp")
    nc.vector.tensor_copy(out=rtmp[:], in_=ctxp[64:65, :])
    rt = rpool.tile([1, 512], FP32, tag="rt")
    nc.vector.reciprocal_approx_fast(out=rt[:], in_=rtmp[:])
    rb = rpool.tile([64, 512], FP32, tag="rb")
    nc.gpsimd.partition_broadcast(rb[:], rt[0:1, :], channels=64)
    nc.vector.tensor_mul(ctx_sb[hb:hb + 64, hc, qsl], ctxp[0:64, :], rb[:])


def _body(ctx, nc, tc, xt_d, wq_d, wk_d, wv_d, wo_d, out_d):
    xt_r = xt_d.rearrange("(f p) t -> p f t", p=128)
    persist = ctx.enter_context(tc.tile_pool(name="persist", bufs=1))
    xbig = persist.tile([128, FC, T], BF16, tag="x")
    qT = persist.tile([128, 4, T], BF16, tag="qT")
    kT = persist.tile([128, 4, T], BF16, tag="kT")
    v_sb = persist.tile([128, KC, HPC, 65], BF16, tag="v")
    ctx_sb = persist.tile([128, 4, T], BF16, tag="ctx")
    wq_sb = persist.tile([128, FC, 512], BF16, tag="wq")
    wk_sb = persist.tile([128, FC, 512], BF16, tag="wk")
    wv_sb = persist.tile([128, FC, 512], BF16, tag="wv")
    wo_sb = persist.tile([128, 4, D], BF16, tag="wo")
    # circular exp-output buffer, indexed by global chunk mod NP2; reuse
    # distance (20 chunks) far exceeds the AV defer + quarter-trailing window.
    P2big = persist.tile([128, 20, 2, 512], BF16, tag="P2big")
    warm = persist.tile([1, 4], FP32, tag="warm")

    # Preload the ACT exp table-set during the initial DMA wait.
    nc.vector.memset(warm[:], 0.0)
    nc.scalar.activation(out=warm[:], in_=warm[:], func=EXP)

    nc.vector.memset(v_sb[:, :, :, 64:65], 1.0)

    # DMA in first-use order: wq+x0 (span-0 q proj), wk (span-0 k proj),
    # wv lo-half (v heads 0-3 JIT in the first quarter), x spans 1-3
    # (k-span proj at chunks 0-11), wv hi-half, wout (used from g=208).
    nc.sync.dma_start(out=wq_sb[:], in_=wq_d.rearrange("(f p) c -> p f c", p=128))
    nc.sync.dma_start(out=xbig[:, :, 0:512], in_=xt_r[:, :, 0:512])
    nc.sync.dma_start(out=wk_sb[:], in_=wk_d.rearrange("(f p) c -> p f c", p=128))
    wv_r = wv_d.rearrange("(f p) c -> p f c", p=128)
    nc.sync.dma_start(out=wv_sb[:, :, 0:256], in_=wv_r[:, :, 0:256])
    nc.sync.dma_start(out=xbig[:, :, 512:1024], in_=xt_r[:, :, 512:1024])
    nc.sync.dma_start(out=xbig[:, :, 1024:1536], in_=xt_r[:, :, 1024:1536])
    nc.sync.dma_start(out=wv_sb[:, :, 256:512], in_=wv_r[:, :, 256:512])
    nc.sync.dma_start(out=xbig[:, :, 1536:2048], in_=xt_r[:, :, 1536:2048])
    nc.sync.dma_start(out=wo_sb[:], in_=wo_d.rearrange("(c p) d -> p c d", p=128))

    spsum = ctx.enter_context(tc.tile_pool(name="spsum", bufs=2, space="PSUM"))
    cpsum = ctx.enter_context(tc.tile_pool(name="cpsum", bufs=2, space="PSUM"))
    # 8 PSUM banks total: spsum 4, ctx ring 2, unified projection ring 2.
    ps_u = ctx.enter_context(tc.tile_pool(name="ps_u", bufs=2, space="PSUM"))
    rpool = ctx.enter_context(tc.tile_pool(name="rpool", bufs=2))
    osb = ctx.enter_context(tc.tile_pool(name="osb", bufs=2))

    # ---- projection units ----
    # Every unit is atomic within one chunk (<=860ns of PE), accumulates in a
    # [128,256] fp32 slot of the shared 2-bank ring, and ends with one copy.

    def qk_burst(pair, ts, qk):
        """Immediate full-span projection of one (pair, span, q-or-k)."""
        tsl = slice(ts * 512, (ts + 1) * 512)
        w_sb, dst = ((wq_sb, qT), (wk_sb, kT))[qk]
        p = ps_u.tile([128, 512], FP32, tag="acc", name="qkburst")
        for fc in range(FC):
            nc.tensor.matmul(
                p[:],
                lhsT=w_sb[:, fc, pair * 128:(pair + 1) * 128],
                rhs=xbig[:, fc, tsl],
                start=(fc == 0), stop=(fc == FC - 1))
        nc.vector.tensor_copy(out=dst[:, pair, tsl], in_=p[:])

    def qk_unit(pair, ts, qk, half):
        """Projection of 256 tokens of one (pair, span, q-or-k)."""
        tsl = slice(ts * 512 + half * 256, ts * 512 + half * 256 + 256)
        w_sb, dst = ((wq_sb, qT), (wk_sb, kT))[qk]
        p = ps_u.tile([128, 512], FP32, tag="acc", name="qkp")
        for fc in range(FC):
            nc.tensor.matmul(
                p[:, 0:256],
                lhsT=w_sb[:, fc, pair * 128:(pair + 1) * 128],
                rhs=xbig[:, fc, tsl],
                start=(fc == 0), stop=(fc == FC - 1))
        nc.vector.tensor_copy(out=dst[:, pair, tsl], in_=p[:, 0:256])

    def v_unit(kc, lo):
        """v projection for k-chunk kc, heads 0-3 (lo) or 4-7 (hi)."""
        csl = slice(0, 256) if lo else slice(256, 512)
        psv = ps_u.tile([128, 512], FP32, tag="acc", name="psv")
        xc = slice((kc // 4) * 512 + (kc % 4) * 128,
                   (kc // 4) * 512 + (kc % 4) * 128 + 128)
        for fc in range(FC):
            nc.tensor.matmul(
                psv[:, 0:256],
                lhsT=xbig[:, fc, xc],
                rhs=wv_sb[:, fc, csl],
                start=(fc == 0), stop=(fc == FC - 1))
        h0 = 0 if lo else 4
        nc.vector.tensor_copy(
            out=v_sb[:, kc, h0:h0 + 4, 0:64],
            in_=psv[:, 0:256].rearrange("p (h d) -> p h d", h=4))

    def make_op_unit(qq_prev):
        """4-cc output-projection unit for 128 tokens x 256 columns."""
        st = {"ot": None}

        def unit(s):
            unit_i, colq = divmod(s, 4)
            tcg = qq_prev * 4 + unit_i
            if colq == 0:
                st["ot"] = osb.tile([128, D], FP32, tag="ot", name="ot")
            po = ps_u.tile([128, 512], FP32, tag="acc", name="po")
            csl = slice(colq * 256, (colq + 1) * 256)
            for cc in range(4):
                nc.tensor.matmul(
                    po[:, 0:256],
                    lhsT=ctx_sb[:, cc, tcg * 128:(tcg + 1) * 128],
                    rhs=wo_sb[:, cc, csl],
                    start=(cc == 0), stop=(cc == 3))
            nc.vector.tensor_copy(out=st["ot"][:, csl], in_=po[:, 0:256])
            if colq == 3:
                nc.sync.dma_start(
                    out=out_d[tcg * 128:(tcg + 1) * 128, :],
                    in_=st["ot"][:])
        return unit

    # ---- schedule: extra PE work per global chunk g ----

    sched = {g: [] for g in range(NG)}

    def put(g, fn, *args):
        sched[g].append((fn, args))

    # v heads 0-3: JIT at chunk kc (AV(kc) is emitted at chunk kc+AVD).
    for kc in range(KC):
        put(kc, v_unit, kc, True)
    # pair 0 remaining spans as half-span units, in need-order:
    # k1 by g4, k2 by g8, k3 by g12, q1 by g16, q2 by g32, q3 by g48.
    p0_units = [(1, 1), (2, 1), (3, 1), (1, 0)]
    for u, (ts, qk) in enumerate(p0_units):
        for half in range(2):
            put(2 * u + half, qk_unit, 0, ts, qk, half)
    for u, ts in enumerate((2, 3)):
        for half in range(2):
            put(16 + 2 * u + half, qk_unit, 0, ts, 0, half)
    # v heads 4-7 (needed by g=64) and pairs 1-3 (k0..k3 then q0..q3,
    # finishing 8 chunks before each pair's first attention chunk).
    for kc in range(KC):
        put(21 + 2 * kc, v_unit, kc, False)
    for pair in range(1, 4):
        units = [(ts, 1) for ts in range(4)] + [(ts, 0) for ts in range(4)]
        g0 = 64 * pair - 42 if pair > 1 else 22
        for u, (ts, qk) in enumerate(units):
            for half in range(2):
                put(g0 + 2 * (2 * u + half), qk_unit, pair, ts, qk, half)
    # output projection: quarter Q's 16 units after quarter Q's final norm
    # (norm of hc3's quarter Q lands at g = 16*(12+Q)+18).
    for q in range(3):
        opu = make_op_unit(q)
        for s in range(16):
            put(min(211 + 16 * q + s, 255), opu, s)

    # ---- the flat attention pipeline ----

    qk_burst(0, 0, 0)
    qk_burst(0, 0, 1)

    qinfo = {}

    def emit_av(j):
        info = qinfo[j // 16]
        kc = j % 16
        for i, ctxp in ((0, info["ctxA"]), (1, info["ctxB"])):
            nc.tensor.matmul(
                ctxp[:],
                lhsT=v_sb[:, kc, 2 * info["hc"] + i, :],
                rhs=P2big[:, j % 20, i, :],
                start=(kc == 0), stop=(kc == KC - 1))

    def emit_norm(q):
        info = qinfo[q]
        qsl = slice(info["qq"] * 512, (info["qq"] + 1) * 512)
        _norm(nc, rpool, ctx_sb, info["ctxA"], 2 * info["hc"], info["hc"], qsl)
        _norm(nc, rpool, ctx_sb, info["ctxB"], 2 * info["hc"] + 1,
              info["hc"], qsl)
        del qinfo[q]

    for g in range(NG):
        hc, qq, kc = g // 64, (g // 16) % 4, g % 16
        if kc == 0:
            qinfo[g // 16] = {
                "hc": hc, "qq": qq,
                "ctxA": cpsum.tile([65, 512], FP32, tag="ctx", name="ctxA"),
                "ctxB": cpsum.tile([65, 512], FP32, tag="ctx", name="ctxB"),
            }
        qsl = slice(qq * 512, (qq + 1) * 512)
        sps = spsum.tile([128, 2, 512], FP32, tag="S")
        for i in range(2):          # head A on rows 0-63, head B on 64-127
            b0 = i * 64
            nc.tensor.matmul(
                sps[:, i, :],
                lhsT=kT[b0:b0 + 64, hc, kc * 128:(kc + 1) * 128],
                rhs=qT[b0:b0 + 64, hc, qsl],
                start=True, stop=True)
        nc.scalar.activation(
            out=P2big[:, g % 20, :, :], in_=sps[:], func=EXP, scale=0.125)
        for fn, args in sched[g]:
            fn(*args)
        if g >= AVD:
            emit_av(g - AVD)
            if (g - AVD) % 16 == 15:
                emit_norm((g - AVD) // 16)
    for j in range(NG - AVD, NG):
        emit_av(j)
    emit_norm(15)
    # tail: output projection for the last quarter
    opu = make_op_unit(3)
    for s in range(16):
        opu(s)


def build():
    nc = bacc.Bacc("TRN2", target_bir_lowering=False, debug=False, num_devices=8)
    xt_d = nc.dram_tensor("xt", [D, T], BF16, kind="ExternalInput").ap()
    wq_d = nc.dram_tensor("wq", [D, 512], BF16, kind="ExternalInput").ap()
    wk_d = nc.dram_tensor("wk", [D, 512], BF16, kind="ExternalInput").ap()
    wv_d = nc.dram_tensor("wv", [D, 512], BF16, kind="ExternalInput").ap()
    wo_d = nc.dram_tensor("wout", [512, D], BF16, kind="ExternalInput").ap()
    out_d = nc.dram_tensor("out", [T, D], FP32, kind="ExternalOutput").ap()
    with tile.TileContext(nc) as tc:
        with ExitStack() as ctx:
            _body(ctx, nc, tc, xt_d, wq_d, wk_d, wv_d, wo_d, out_d)
    nc.compile()
    return nc


_nc = None


def _get_nc():
    global _nc
    if _nc is None:
        _nc = build()
    return _nc


def make_in_maps(x, Wqkv, Wout):
    bf = ml_dtypes.bfloat16
    in_maps = []
    for c in range(8):
        b, g = divmod(c, 2)
        cs = slice(g * 512, (g + 1) * 512)
        in_maps.append({
            "xt": np.ascontiguousarray(x[b].T).astype(bf),
            "wq": np.ascontiguousarray(Wqkv[:, 0 * D:1 * D][:, cs]).astype(bf),
            "wk": np.ascontiguousarray(Wqkv[:, 1 * D:2 * D][:, cs]).astype(bf),
            "wv": np.ascontiguousarray(Wqkv[:, 2 * D:3 * D][:, cs]).astype(bf),
            "wout": np.ascontiguousarray(Wout[cs, :]).astype(bf),
        })
    return in_maps


def kernel(x, Wqkv, Wout, _trace=False):
    nc = _get_nc()
    x = np.asarray(x, dtype=np.float32)
    Wqkv = np.asarray(Wqkv, dtype=np.float32)
    Wout = np.asarray(Wout, dtype=np.float32)
    in_maps = make_in_maps(x, Wqkv, Wout)
    kwargs = {}
    if _trace:
        kwargs["trace"] = True
    res = run_bass_kernel_spmd(nc, in_maps, core_ids=list(range(8)), **kwargs)
    outs = [res.results[c]["out"] for c in range(8)]
    out = np.stack([outs[2 * b] + outs[2 * b + 1] for b in range(4)])
    if _trace:
        kernel.last_result = res
    return out


# revision 11
# speedup vs baseline: 1.2298x; 1.0138x over previous
"""Multi-head attention (B=4, T=2048, D=1024, H=16) on 8 TRN2 NeuronCores.

Sharding: core c -> (batch b = c//2, head-group g = c%2 of 8 heads).
Each core computes the qkv projection for its batch restricted to its 8
heads, full attention for those heads, and a partial output projection
(ctx_local @ Wout[rows of its heads]).  Host sums the two partials per batch.

Per-core kernel: a single flat software pipeline over 256 attention chunks
(4 head-pairs x 4 query-quarters x 16 k-chunks).  Per chunk: an S pair (two
concurrent 64-row-tile matmuls, one per head), exp on ACT, and the AV pair
deferred 3 chunks.  All projection work (qk for later pairs, v, output) is
broken into ~215ns steps and scheduled into specific chunks so the PE stream
stays ahead of ACT, which is the binding engine (~1us per chunk).  x stays
resident in SBUF; DMAs are issued in first-use order so attention starts
~11us in instead of ~43us.
"""

import numpy as np
import ml_dtypes
from contextlib import ExitStack

import concourse.bass as bass
import concourse.bacc as bacc
import concourse.tile as tile
from concourse import mybir
from concourse.bass_utils import run_bass_kernel_spmd

FP32 = mybir.dt.float32
BF16 = mybir.dt.bfloat16
EXP = mybir.ActivationFunctionType.Exp

D = 1024
T = 2048
HPC = 8          # heads per core
FC = 8           # feature chunks of 128 (projection contraction)
KC = 16          # k chunks of 128 per quarter
NG = 256         # total chunks: 4 pairs x 4 quarters x 16
AVD = 3          # AV defer (chunks)


def _norm(nc, rpool, ctx_sb, ctxp, hh, hc, qsl):
    """ctx_sb[hb:hb+64, hc, qsl] = ctxp[0:64] / ctxp[64] (sumexp row)."""
    hb = (hh % 2) * 64
    rtmp = rpool.tile([1, 512], FP32, tag="rtmp")
    nc.vector.tensor_copy(out=rtmp[:], in_=ctxp[64:65, :])
    rt = rpool.tile([1, 512], FP32, tag="rt")
    nc.vector.reciprocal_approx_fast(out=rt[:], in_=rtmp[:])
    rb = rpool.tile([64, 512], FP32, tag="rb")
    nc.gpsimd.partition_broadcast(rb[:], rt[0:1, :], channels=64)
    nc.vector.tensor_mul(ctx_sb[hb:hb + 64, hc, qsl], ctxp[0:64, :], rb[:])


def _body(ctx, nc, tc, xt_d, wq_d, wk_d, wv_d, wo_d, out_d):
    xt_r = xt_d.rearrange("(f p) t -> p f t", p=128)
    persist = ctx.enter_context(tc.tile_pool(name="persist", bufs=1))
    xbig = persist.tile([128, FC, T], BF16, tag="x")
    qT = persist.tile([128, 4, T], BF16, tag="qT")
    kT = persist.tile([128, 4, T], BF16, tag="kT")
    v_sb = persist.tile([128, KC, HPC, 65], BF16, tag="v")
    ctx_sb = persist.tile([128, 4, T], BF16, tag="ctx")
    wq_sb = persist.tile([128, FC, 512], BF16, tag="wq")
    wk_sb = persist.tile([128, FC, 512], BF16, tag="wk")
    wv_sb = persist.tile([128, FC, 512], BF16, tag="wv")
    wo_sb = persist.tile([128, 4, D], BF16, tag="wo")
    # circular exp-output buffer, indexed by global chunk mod NP2; reuse
    # distance (20 chunks) far exceeds the AV defer + quarter-trailing window.
    P2big = persist.tile([128, 20, 2, 512], BF16, tag="P2big")
    warm = persist.tile([1, 4], FP32, tag="warm")

    # Preload the ACT exp table-set during the initial DMA wait.
    nc.vector.memset(warm[:], 0.0)
    nc.scalar.activation(out=warm[:], in_=warm[:], func=EXP)

    nc.vector.memset(v_sb[:, :, :, 64:65], 1.0)

    # DMA in first-use order: wq+x0 (span-0 q proj), wk (span-0 k proj),
    # wv lo-half (v heads 0-3 JIT in the first quarter), x spans 1-3
    # (k-span proj at chunks 0-11), wv hi-half, wout (used from g=208).
    nc.sync.dma_start(out=wq_sb[:], in_=wq_d.rearrange("(f p) c -> p f c", p=128))
    nc.sync.dma_start(out=xbig[:, :, 0:512], in_=xt_r[:, :, 0:512])
    nc.sync.dma_start(out=wk_sb[:], in_=wk_d.rearrange("(f p) c -> p f c", p=128))
    wv_r = wv_d.rearrange("(f p) c -> p f c", p=128)
    nc.sync.dma_start(out=wv_sb[:, :, 0:256], in_=wv_r[:, :, 0:256])
    nc.sync.dma_start(out=xbig[:, :, 512:1024], in_=xt_r[:, :, 512:1024])
    nc.sync.dma_start(out=xbig[:, :, 1024:1536], in_=xt_r[:, :, 1024:1536])
    nc.sync.dma_start(out=wv_sb[:, :, 256:512], in_=wv_r[:, :, 256:512])
    nc.sync.dma_start(out=xbig[:, :, 1536:2048], in_=xt_r[:, :, 1536:2048])
    nc.sync.dma_start(out=wo_sb[:], in_=wo_d.rearrange("(c p) d -> p c d", p=128))

    spsum = ctx.enter_context(tc.tile_pool(name="spsum", bufs=2, space="PSUM"))
    cpsum = ctx.enter_context(tc.tile_pool(name="cpsum", bufs=2, space="PSUM"))
    # 8 PSUM banks total: spsum 4, ctx ring 2, unified projection ring 2.
    ps_u = ctx.enter_context(tc.tile_pool(name="ps_u", bufs=2, space="PSUM"))
    rpool = ctx.enter_context(tc.tile_pool(name="rpool", bufs=2))
    osb = ctx.enter_context(tc.tile_pool(name="osb", bufs=2))

    # ---- projection units ----
    # Every unit is atomic within one chunk (<=860ns of PE), accumulates in a
    # [128,256] fp32 slot of the shared 2-bank ring, and ends with one copy.

    def qk_burst(pair, ts, qk):
        """Immediate full-span projection of one (pair, span, q-or-k)."""
        tsl = slice(ts * 512, (ts + 1) * 512)
        w_sb, dst = ((wq_sb, qT), (wk_sb, kT))[qk]
        p = ps_u.tile([128, 512], FP32, tag="acc", name="qkburst")
        for fc in range(FC):
            nc.tensor.matmul(
                p[:],
                lhsT=w_sb[:, fc, pair * 128:(pair + 1) * 128],
                rhs=xbig[:, fc, tsl],
                start=(fc == 0), stop=(fc == FC - 1))
        nc.vector.tensor_copy(out=dst[:, pair, tsl], in_=p[:])

    qk_acc = {}

    def qk_half(pair, ts, qk, half, fchalf):
        """4-fc half of a 256-token projection; accumulator spans 2 chunks."""
        tsl = slice(ts * 512 + half * 256, ts * 512 + half * 256 + 256)
        w_sb, dst = ((wq_sb, qT), (wk_sb, kT))[qk]
        key = (pair, ts, qk, half)
        if fchalf == 0:
            qk_acc[key] = ps_u.tile([128, 512], FP32, tag="acc", name="qkp")
        p = qk_acc.pop(key) if fchalf == 1 else qk_acc[key]
        for fc in range(4 * fchalf, 4 * fchalf + 4):
            nc.tensor.matmul(
                p[:, 0:256],
                lhsT=w_sb[:, fc, pair * 128:(pair + 1) * 128],
                rhs=xbig[:, fc, tsl],
                start=(fc == 0), stop=(fc == FC - 1))
        if fchalf == 1:
            nc.vector.tensor_copy(out=dst[:, pair, tsl], in_=p[:, 0:256])

    v_acc = {}

    def v_unit(kc, lo):
        """v projection for k-chunk kc, heads 0-3 (lo), single chunk."""
        psv = ps_u.tile([128, 512], FP32, tag="acc", name="psv")
        xc = slice((kc // 4) * 512 + (kc % 4) * 128,
                   (kc // 4) * 512 + (kc % 4) * 128 + 128)
        for fc in range(FC):
            nc.tensor.matmul(
                psv[:, 0:256],
                lhsT=xbig[:, fc, xc],
                rhs=wv_sb[:, fc, 0:256],
                start=(fc == 0), stop=(fc == FC - 1))
        nc.vector.tensor_copy(
            out=v_sb[:, kc, 0:4, 0:64],
            in_=psv[:, 0:256].rearrange("p (h d) -> p h d", h=4))

    def v_half(kc, fchalf):
        """4-fc half of the heads-4-7 v projection for k-chunk kc."""
        if fchalf == 0:
            v_acc[kc] = ps_u.tile([128, 512], FP32, tag="acc", name="psv")
        psv = v_acc.pop(kc) if fchalf == 1 else v_acc[kc]
        xc = slice((kc // 4) * 512 + (kc % 4) * 128,
                   (kc // 4) * 512 + (kc % 4) * 128 + 128)
        for fc in range(4 * fchalf, 4 * fchalf + 4):
            nc.tensor.matmul(
                psv[:, 0:256],
                lhsT=xbig[:, fc, xc],
                rhs=wv_sb[:, fc, 256:512],
                start=(fc == 0), stop=(fc == FC - 1))
        if fchalf == 1:
            nc.vector.tensor_copy(
                out=v_sb[:, kc, 4:8, 0:64],
                in_=psv[:, 0:256].rearrange("p (h d) -> p h d", h=4))

    def make_op_unit(qq_prev):
        """2-cc half-units of the output projection of qq_prev's tokens.
        s in 0..31: (tcg, colq) unit = s//2, cc pair = s%2."""
        st = {"ot": None, "po": None}

        def unit(s):
            u, cchalf = divmod(s, 2)
            unit_i, colq = divmod(u, 4)
            tcg = qq_prev * 4 + unit_i
            csl = slice(colq * 256, (colq + 1) * 256)
            if cchalf == 0:
                if colq == 0:
                    st["ot"] = osb.tile([128, D], FP32, tag="ot", name="ot")
                st["po"] = ps_u.tile([128, 512], FP32, tag="acc", name="po")
            for cc in (0, 1) if cchalf == 0 else (2, 3):
                nc.tensor.matmul(
                    st["po"][:, 0:256],
                    lhsT=ctx_sb[:, cc, tcg * 128:(tcg + 1) * 128],
                    rhs=wo_sb[:, cc, csl],
                    start=(cc == 0), stop=(cc == 3))
            if cchalf == 1:
                nc.vector.tensor_copy(out=st["ot"][:, csl], in_=st["po"][:, 0:256])
                if colq == 3:
                    nc.sync.dma_start(
                        out=out_d[tcg * 128:(tcg + 1) * 128, :],
                        in_=st["ot"][:])
        return unit

    # ---- schedule    # ---- schedule: extra PE work per global chunk g ----

    sched = {g: [] for g in range(NG)}

    def put(g, fn, *args):
        sched[min(g, NG - 1)].append((fn, args))

    def put_span(g0, g1, items):
        """Spread items evenly over chunks [g0, g1]."""
        n, w = len(items), g1 - g0 + 1
        for i, (fn, args) in enumerate(items):
            put(g0 + i * w // n, fn, *args)

    def qk_halves(pair, units):
        return [(qk_half, (pair, ts, qk, half, fch))
                for ts, qk in units for half in range(2) for fch in range(2)]

    # v heads 0-3: JIT at chunk kc (AV(kc) is emitted at chunk kc+AVD).
    for kc in range(KC):
        put(kc, v_unit, kc, True)
    # pair 0 remaining spans: k1 by g4, k2 by g8, k3 by g12, q1 by g16,
    # q2 by g32, q3 by g48 -- 4 half-steps per (span, q/k), 1 per chunk.
    put_span(0, 3, qk_halves(0, [(1, 1)]))
    put_span(4, 7, qk_halves(0, [(2, 1)]))
    put_span(8, 11, qk_halves(0, [(3, 1)]))
    put_span(12, 15, qk_halves(0, [(1, 0)]))
    put_span(16, 19, qk_halves(0, [(2, 0)]))
    put_span(20, 23, qk_halves(0, [(3, 0)]))
    # v heads 4-7 (needed by g64) interleaved with pair 1's k and q0/q1.
    put_span(24, 55, [(v_half, (kc, fch)) for kc in range(KC) for fch in range(2)])
    put_span(16, 47, qk_halves(1, [(ts, 1) for ts in range(4)]))
    put_span(48, 57, qk_halves(1, [(0, 0)]) + qk_halves(1, [(1, 0)])[:2])
    put_span(58, 70, qk_halves(1, [(1, 0)])[2:] + qk_halves(1, [(2, 0)]))
    put_span(71, 90, qk_halves(1, [(3, 0)]))
    # pairs 2 and 3 spread across their full windows.
    put_span(64, 119, qk_halves(2, [(ts, 1) for ts in range(4)] +
                                   [(0, 0), (1, 0)]))
    put_span(120, 140, qk_halves(2, [(2, 0), (3, 0)]))
    put_span(128, 183, qk_halves(3, [(ts, 1) for ts in range(4)] +
                                    [(0, 0), (1, 0)]))
    put_span(184, 204, qk_halves(3, [(2, 0), (3, 0)]))
    # output projection: quarter Q's half-units after norm(12+Q) at ~16Q+218.
    for q in range(3):
        opu = make_op_unit(q)
        put_span(211 + 16 * q, min(210 + 16 * (q + 1), 255),
                 [(opu, (s,)) for s in range(32)])

    # ---- the flat attention pipeline ----

    qk_burst(0, 0, 0)
    qk_burst(0, 0, 1)

    qinfo = {}

    def emit_av(j):
        info = qinfo[j // 16]
        kc = j % 16
        for i, ctxp in ((0, info["ctxA"]), (1, info["ctxB"])):
            nc.tensor.matmul(
                ctxp[:],
                lhsT=v_sb[:, kc, 2 * info["hc"] + i, :],
                rhs=P2big[:, j % 20, i, :],
                start=(kc == 0), stop=(kc == KC - 1))

    def emit_norm(q):
        info = qinfo[q]
        qsl = slice(info["qq"] * 512, (info["qq"] + 1) * 512)
        _norm(nc, rpool, ctx_sb, info["ctxA"], 2 * info["hc"], info["hc"], qsl)
        _norm(nc, rpool, ctx_sb, info["ctxB"], 2 * info["hc"] + 1,
              info["hc"], qsl)
        del qinfo[q]

    for g in range(NG):
        hc, qq, kc = g // 64, (g // 16) % 4, g % 16
        if kc == 0:
            qinfo[g // 16] = {
                "hc": hc, "qq": qq,
                "ctxA": cpsum.tile([65, 512], FP32, tag="ctx", name="ctxA"),
                "ctxB": cpsum.tile([65, 512], FP32, tag="ctx", name="ctxB"),
            }
        qsl = slice(qq * 512, (qq + 1) * 512)
        sps = spsum.tile([128, 2, 512], FP32, tag="S")
        for i in range(2):          # head A on rows 0-63, head B on 64-127
            b0 = i * 64
            nc.tensor.matmul(
                sps[:, i, :],
                lhsT=kT[b0:b0 + 64, hc, kc * 128:(kc + 1) * 128],
                rhs=qT[b0:b0 + 64, hc, qsl],
                start=True, stop=True)
        nc.scalar.activation(
            out=P2big[:, g % 20, :, :], in_=sps[:], func=EXP, scale=0.125)
        for fn, args in sched[g]:
            fn(*args)
        if g >= AVD:
            emit_av(g - AVD)
            if (g - AVD) % 16 == 15:
                emit_norm((g - AVD) // 16)
    for j in range(NG - AVD, NG):
        emit_av(j)
    emit_norm(15)
    # tail: output projection for the last quarter
    opu = make_op_unit(3)
    for s in range(32):
        opu(s)


def build():
    nc = bacc.Bacc("TRN2", target_bir_lowering=False, debug=False, num_devices=8)
    xt_d = nc.dram_tensor("xt", [D, T], BF16, kind="ExternalInput").ap()
    wq_d = nc.dram_tensor("wq", [D, 512], BF16, kind="ExternalInput").ap()
    wk_d = nc.dram_tensor("wk", [D, 512], BF16, kind="ExternalInput").ap()
    wv_d = nc.dram_tensor("wv", [D, 512], BF16, kind="ExternalInput").ap()
    wo_d = nc.dram_tensor("wout", [512, D], BF16, kind="ExternalInput").ap()
    out_d = nc.dram_tensor("out", [T, D], FP32, kind="ExternalOutput").ap()
    with tile.TileContext(nc) as tc:
        with ExitStack() as ctx:
            _body(ctx, nc, tc, xt_d, wq_d, wk_d, wv_d, wo_d, out_d)
    nc.compile()
    return nc


_nc = None


def _get_nc():
    global _nc
    if _nc is None:
        _nc = build()
    return _nc


def make_in_maps(x, Wqkv, Wout):
    bf = ml_dtypes.bfloat16
    in_maps = []
    for c in range(8):
        b, g = divmod(c, 2)
        cs = slice(g * 512, (g + 1) * 512)
        in_maps.append({
            "xt": np.ascontiguousarray(x[b].T).astype(bf),
            "wq": np.ascontiguousarray(Wqkv[:, 0 * D:1 * D][:, cs]).astype(bf),
            "wk": np.ascontiguousarray(Wqkv[:, 1 * D:2 * D][:, cs]).astype(bf),
            "wv": np.ascontiguousarray(Wqkv[:, 2 * D:3 * D][:, cs]).astype(bf),
            "wout": np.ascontiguousarray(Wout[cs, :]).astype(bf),
        })
    return in_maps


def kernel(x, Wqkv, Wout, _trace=False):
    nc = _get_nc()
    x = np.asarray(x, dtype=np.float32)
    Wqkv = np.asarray(Wqkv, dtype=np.float32)
    Wout = np.asarray(Wout, dtype=np.float32)
    in_maps = make_in_maps(x, Wqkv, Wout)
    kwargs = {}
    if _trace:
        kwargs["trace"] = True
    res = run_bass_kernel_spmd(nc, in_maps, core_ids=list(range(8)), **kwargs)
    outs = [res.results[c]["out"] for c in range(8)]
    out = np.stack([outs[2 * b] + outs[2 * b + 1] for b in range(4)])
    if _trace:
        kernel.last_result = res
    return out
